# revision 4
# baseline (speedup 1.0000x reference)
"""Multi-head attention (B=2, S=2048, D=1024, H=16) on 8 TRN2 NeuronCores.

Sharding: tensor-parallel over heads (TP=4, 4 heads / 256 dims per core)
x data-parallel over batch (DP=2). Core c = 4*b + t handles batch b,
head group t. Each core computes Q/K/V projections for its heads,
attention in a transposed-scores layout (scores^T = [s_k, s_q], softmax
across partitions via a ones-column appended to V and a K=1 outer-product
broadcast of the reciprocal), then its partial output projection.
Partials are ReduceScattered over each batch's 4-core TP group; the host
reassembles the full [B, S, D] output.

All matmul operands are bf16 (fp32 PSUM accumulation); softmax
denominators/reciprocals and the output path are fp32. The key mask is
folded into the exp as a per-partition bias (0 or -60).
"""

import contextlib
import numpy as np
import ml_dtypes

import concourse.bass as bass
import concourse.tile as tile
from concourse import bacc, mybir
from concourse.bass_utils import run_bass_kernel_spmd

F32 = mybir.dt.float32
BF16 = mybir.dt.bfloat16
Exp = mybir.ActivationFunctionType.Exp

B, S, D, H = 2, 2048, 1024, 16
DK = D // H                      # 64
TP, DP = 4, 2
HPC = H // TP                    # heads per core = 4
DSH = D // TP                    # shard dims per core = 256
NPAIR = HPC // 2                 # head pairs per core = 2
QB = 512                         # query block
NQB = S // QB                    # 4
KT = 128                         # key tile
NKT = S // KT                    # 16
NKB = D // 128                   # 8 contraction tiles for projections
MASK_NEG = -60.0

REPLICA_GROUPS = [[0, 1, 2, 3], [4, 5, 6, 7]]


def build_nc():
    nc = bacc.Bacc("TRN2", target_bir_lowering=False, debug=False, num_devices=DP * TP)

    # ---- parameters (per-core shards, host-prepped layouts)
    xq = nc.declare_dram_parameter("xq", [NKB, 128, S], BF16, isOutput=False)   # q_in[b].T
    xk = nc.declare_dram_parameter("xk", [NKB, 128, S], BF16, isOutput=False)
    xv = nc.declare_dram_parameter("xv", [NKB, 128, S], BF16, isOutput=False)
    wq = nc.declare_dram_parameter("wq", [NKB, 128, DSH], BF16, isOutput=False)  # w_q[shard].T
    wk = nc.declare_dram_parameter("wk", [NKB, 128, DSH], BF16, isOutput=False)
    wv = nc.declare_dram_parameter("wv", [NKB, 128, DSH], BF16, isOutput=False)
    wo = nc.declare_dram_parameter("wo", [2, 128, D], BF16, isOutput=False)      # w_o[:, shard].T
    bq = nc.declare_dram_parameter("bq", [2, 128], F32, isOutput=False)          # b_q shard
    bk = nc.declare_dram_parameter("bk", [2, 128], F32, isOutput=False)
    bvb = nc.declare_dram_parameter("bvb", [128, DSH], F32, isOutput=False)      # b_v shard bcast
    bob = nc.declare_dram_parameter("bob", [128, D], F32, isOutput=False)        # b_o bcast
    mb = nc.declare_dram_parameter("mb", [128, NKT], F32, isOutput=False)        # mask bias
    out = nc.declare_dram_parameter("out", [NQB, 128, D], F32, isOutput=True)

    with tile.TileContext(nc) as tc, contextlib.ExitStack() as ctx:
        const = ctx.enter_context(tc.tile_pool(name="const", bufs=1))
        xp = ctx.enter_context(tc.tile_pool(name="xp", bufs=NKB))
        qt_p = ctx.enter_context(tc.tile_pool(name="qt", bufs=2 * NQB))
        kt_p = ctx.enter_context(tc.tile_pool(name="ktp", bufs=2 * NQB))
        vp_p = ctx.enter_context(tc.tile_pool(name="vp", bufs=NKT))
        exp_p = ctx.enter_context(tc.tile_pool(name="expp", bufs=3))
        ctx_p = ctx.enter_context(tc.tile_pool(name="ctxp", bufs=4))
        rec_p = ctx.enter_context(tc.tile_pool(name="recp", bufs=2))
        rb_p = ctx.enter_context(tc.tile_pool(name="rbp", bufs=2))
        po_p = ctx.enter_context(tc.tile_pool(name="pop", bufs=2))
        os_p = ctx.enter_context(tc.tile_pool(name="osp", bufs=3))
        ps_s = ctx.enter_context(tc.tile_pool(name="pss", bufs=2, space="PSUM"))
        ps_av = ctx.enter_context(tc.tile_pool(name="psav", bufs=2, space="PSUM"))
        ps_sm = ctx.enter_context(tc.tile_pool(name="pssm", bufs=2, space="PSUM"))
        dram = ctx.enter_context(tc.tile_pool(name="dram", bufs=2, space="DRAM"))

        # ---- constants
        w_sb = {}
        for name, prm, blocks in (("wq", wq, NKB), ("wk", wk, NKB), ("wv", wv, NKB)):
            t = const.tile([128, NKB * DSH], BF16, name=f"{name}_sb")
            for kb in range(blocks):
                nc.sync.dma_start(out=t[:, kb * DSH:(kb + 1) * DSH], in_=prm[kb])
            w_sb[name] = t
        wo_sb = const.tile([128, 2 * D], BF16)
        for kb in range(2):
            nc.sync.dma_start(out=wo_sb[:, kb * D:(kb + 1) * D], in_=wo[kb])
        bq_sb = const.tile([128, 2], F32)
        bk_sb = const.tile([128, 2], F32)
        for m in range(2):
            nc.sync.dma_start(out=bq_sb[:, m], in_=bq[m])
            nc.sync.dma_start(out=bk_sb[:, m], in_=bk[m])
        bvb_sb = const.tile([128, DSH], F32)
        nc.sync.dma_start(out=bvb_sb[:], in_=bvb[:])
        bob_sb = const.tile([128, D], F32)
        nc.sync.dma_start(out=bob_sb[:], in_=bob[:])
        mb_sb = const.tile([128, NKT], F32)
        nc.sync.dma_start(out=mb_sb[:], in_=mb[:])
        ones_sb = const.tile([128, DK], F32)
        nc.any.memset(ones_sb[:], 1.0)

        # ---- phase A: projections
        # K^T and Q^T per (pair m, s-block nb): tiles [128, 512]
        #   partitions 0:64 = head 2m dims, 64:128 = head 2m+1 dims
        # V' per s-tile st: [128, HPC*65] with ones col at 64 of each 65
        KT_t = {}
        QT_t = {}
        VP_t = {}

        def proj_qk(xprm, wname, bias_sb, store, order):
            xt = []
            for kb in range(NKB):
                t = xp.tile([128, S], BF16, name=f"x_{wname}_{kb}", tag="xtile")
                nc.sync.dma_start(out=t[:], in_=xprm[kb])
                xt.append(t)
            for nb in order:
                for m in range(2):
                    ps = ps_sm.tile([128, QB], F32, name=f"ps_{wname}_{m}_{nb}", tag="smps")
                    for kb in range(NKB):
                        nc.tensor.matmul(
                            ps[:],
                            w_sb[wname][:, kb * DSH + m * 128: kb * DSH + (m + 1) * 128],
                            xt[kb][:, nb * QB:(nb + 1) * QB],
                            start=(kb == 0), stop=(kb == NKB - 1),
                        )
                    dst = (qt_p if store is QT_t else kt_p).tile(
                        [128, QB], BF16, name=f"{wname}t_{m}_{nb}", tag="proj")
                    nc.vector.tensor_scalar_add(dst[:], ps[:], bias_sb[:, m:m + 1])
                    store[(m, nb)] = dst

        def proj_v():
            xt = []
            for kb in range(NKB):
                t = xp.tile([128, S], BF16, name=f"x_v_{kb}", tag="xtile")
                nc.sync.dma_start(out=t[:], in_=xv[kb])
                xt.append(t)
            for st in range(NKT):
                ps = ps_sm.tile([128, QB], F32, name=f"ps_v_{st}", tag="smps")[:, 0:DSH]
                for kb in range(NKB):
                    nc.tensor.matmul(
                        ps[:],
                        xt[kb][:, st * 128:(st + 1) * 128],
                        w_sb["wv"][:, kb * DSH:(kb + 1) * DSH],
                        start=(kb == 0), stop=(kb == NKB - 1),
                    )
                vp = vp_p.tile([128, HPC * (DK + 1)], BF16, name=f"vp_{st}", tag="vp")
                for h in range(HPC):
                    col = h * (DK + 1) + DK
                    nc.any.memset(vp[:, col:col + 1], 1.0)
                ps3 = ps.rearrange("p (h d) -> p h d", h=HPC)
                bv3 = bvb_sb.rearrange("p (h d) -> p h d", h=HPC)
                vp3 = vp.rearrange("p (h d) -> p h d", h=HPC)[:, :, 0:DK]
                nc.vector.tensor_add(vp3, ps3, bv3)
                VP_t[st] = vp

        proj_qk(xk, "wk", bk_sb, KT_t, order=range(NQB))
        proj_v()
        proj_qk(xq, "wq", bq_sb, QT_t, order=range(NQB))

        # ---- phase B: attention + output projection + reduce-scatter
        for qb in range(NQB):
            ctx_pair = []
            for m in range(NPAIR):
                av = [ps_av.tile([128, QB], F32, name=f"av_{qb}_{m}_{p}", tag="av")
                      for p in range(2)]
                for kt in range(NKT):
                    nb, co = kt // 4, (kt % 4) * 128
                    pss = ps_s.tile([128, 2 * QB], F32, name=f"pss_{qb}_{m}_{kt}", tag="pss")
                    # head 2m on partitions 0:64, head 2m+1 on 64:128;
                    # different PSUM banks for the two row groups (HW req.)
                    nc.tensor.matmul(
                        pss[:, 0:QB],
                        KT_t[(m, nb)][0:64, co:co + 128],
                        QT_t[(m, qb)][0:64, :],
                        start=True, stop=True)
                    nc.tensor.matmul(
                        pss[:, QB:2 * QB],
                        KT_t[(m, nb)][64:128, co:co + 128],
                        QT_t[(m, qb)][64:128, :],
                        start=True, stop=True)
                    et = exp_p.tile([128, 2 * QB], BF16, name=f"exp_{qb}_{m}_{kt}", tag="exp")
                    nc.scalar.activation(et[:], pss[:], Exp,
                                         bias=mb_sb[:, kt:kt + 1], scale=1.0 / np.sqrt(DK))
                    for p in range(2):
                        h = 2 * m + p
                        nc.tensor.matmul(
                            av[p][0:DK + 1, :],
                            VP_t[kt][:, h * (DK + 1):(h + 1) * (DK + 1)],
                            et[:, p * QB:(p + 1) * QB],
                            start=(kt == 0), stop=(kt == NKT - 1),
                        )
                cpt = ctx_p.tile([128, QB], BF16, name=f"ctx_{qb}_{m}", tag="ctx")
                for p in range(2):
                    rec = rec_p.tile([128, QB], F32, name=f"rec_{qb}_{m}_{p}", tag="rec")
                    nc.vector.reciprocal(rec[64:65, :], av[p][DK:DK + 1, :])
                    rbp = ps_sm.tile([128, QB], F32, name=f"rbp_{qb}_{m}_{p}", tag="smps")
                    nc.tensor.matmul(rbp[0:DK, :], ones_sb[64:65, :],
                                     rec[64:65, :], start=True, stop=True)
                    rbs = rb_p.tile([DK, QB], F32, name=f"rbs_{qb}_{m}_{p}", tag="rbs")
                    nc.vector.tensor_copy(rbs[:], rbp[0:DK, :])
                    nc.vector.tensor_mul(cpt[p * DK:(p + 1) * DK, :], av[p][0:DK, :], rbs[:])
                ctx_pair.append(cpt)

            partial = dram.tile([QB, D], F32, name=f"partial_{qb}", tag="partial")
            for st in range(NQB):
                for dh in range(2):
                    pso = ps_sm.tile([128, 512], F32, name=f"pso_{qb}_{st}_{dh}", tag="smps")
                    for m in range(NPAIR):
                        nc.tensor.matmul(
                            pso[:],
                            ctx_pair[m][:, st * 128:(st + 1) * 128],
                            wo_sb[:, m * D + dh * 512: m * D + (dh + 1) * 512],
                            start=(m == 0), stop=(m == NPAIR - 1),
                        )
                    pos = po_p.tile([128, 512], F32, name=f"pos_{qb}_{st}_{dh}", tag="pos")
                    nc.vector.tensor_copy(pos[:], pso[:])
                    nc.sync.dma_start(
                        out=partial[st * 128:(st + 1) * 128, dh * 512:(dh + 1) * 512],
                        in_=pos[:])

            rs_out = dram.tile([128, D], F32, name=f"rs_{qb}", tag="rs")
            nc.gpsimd.collective_compute(
                "ReduceScatter", mybir.AluOpType.add,
                replica_groups=REPLICA_GROUPS,
                ins=[partial[:].opt()], outs=[rs_out[:].opt()])
            osb = os_p.tile([128, D], F32, name=f"os_{qb}", tag="os")
            nc.sync.dma_start(out=osb[:], in_=rs_out[:])
            fin = os_p.tile([128, D], F32, name=f"fin_{qb}", tag="fin")
            nc.vector.tensor_add(fin[:], osb[:], bob_sb[:])
            nc.sync.dma_start(out=out[qb], in_=fin[:])

    nc.compile()
    return nc


def _prep_inputs(q_in, k_in, v_in, mask, w_q, b_q, w_k, b_k, w_v, b_v, w_o, b_o):
    BF = ml_dtypes.bfloat16
    xq_b, xk_b, xv_b, mb_b = [], [], [], []
    for b in range(B):
        xq_b.append(np.ascontiguousarray(q_in[b].T).astype(BF).reshape(NKB, 128, S))
        xk_b.append(np.ascontiguousarray(k_in[b].T).astype(BF).reshape(NKB, 128, S))
        xv_b.append(np.ascontiguousarray(v_in[b].T).astype(BF).reshape(NKB, 128, S))
        mbias = ((mask[b, 0, 0, :] == 0) * np.float32(MASK_NEG)).astype(np.float32)
        mb_b.append(np.ascontiguousarray(mbias.reshape(NKT, 128).T))
    bob = np.ascontiguousarray(np.broadcast_to(b_o.astype(np.float32), (128, D)))
    in_maps = []
    for c in range(DP * TP):
        b, t = c // TP, c % TP
        sl = slice(DSH * t, DSH * (t + 1))
        in_maps.append({
            "xq": xq_b[b], "xk": xk_b[b], "xv": xv_b[b],
            "wq": np.ascontiguousarray(w_q[sl, :].T).astype(BF).reshape(NKB, 128, DSH),
            "wk": np.ascontiguousarray(w_k[sl, :].T).astype(BF).reshape(NKB, 128, DSH),
            "wv": np.ascontiguousarray(w_v[sl, :].T).astype(BF).reshape(NKB, 128, DSH),
            "wo": np.ascontiguousarray(w_o[:, sl].T).astype(BF).reshape(2, 128, D),
            "bq": b_q[sl].astype(np.float32).reshape(2, 128),
            "bk": b_k[sl].astype(np.float32).reshape(2, 128),
            "bvb": np.ascontiguousarray(
                np.broadcast_to(b_v[sl].astype(np.float32), (128, DSH))),
            "bob": bob,
            "mb": mb_b[b],
        })
    return in_maps


_NC_CACHE = {}


def kernel(q_in, k_in, v_in, mask, w_q, b_q, w_k, b_k, w_v, b_v, w_o, b_o):
    if "nc" not in _NC_CACHE:
        _NC_CACHE["nc"] = build_nc()
    nc = _NC_CACHE["nc"]
    in_maps = _prep_inputs(q_in, k_in, v_in, mask,
                           w_q, b_q, w_k, b_k, w_v, b_v, w_o, b_o)
    res = run_bass_kernel_spmd(nc, in_maps, list(range(DP * TP))).results
    full = np.empty((B, S, D), np.float32)
    for b in range(B):
        for r in range(TP):
            o = res[TP * b + r]["out"]          # [NQB, 128, D]
            for qb in range(NQB):
                row = qb * QB + r * 128
                full[b, row:row + 128] = o[qb]
    return full
